# revision 31
# baseline (speedup 1.0000x reference)
"""TRN2 Bass kernel for nn_CustomQLoRABigNet: 6 blocks x (3 QLoRA linears),
ReLU, residual, LayerNorm. Data-parallel over 8 NeuronCores (4096 rows each).

v3 strategy (vs v2 baseline at 2.58ms):
- All weight prep happens on host: W_eff = (q-8)*s + lb@la computed in fp32
  and rounded ONCE to fp16. No dequant / LoRA-fold work on device at all
  (removes 288 fold matmuls + ~430 vector ops + 75MB scales DMA per core).
- fp16 activations/weights everywhere (same PE rate as bf16, 4x less
  rounding error -> large accuracy margin vs the 2e-2 gate).
- Pass/strip-major loop: 3 passes x 6 resident layers (96KB/partition).
  Within a pass each strip of 512 rows flows through all 6 layers using
  two scratch tiles (tA/tB) and an in-place carry tile (tIN) that holds
  the residual; no snapshot copies, no DRAM residual round-trips.
- Strips pipelined in groups of 4; the LayerNorm finish (stats chain,
  rank-1 broadcast matmuls, apply) for stage i is emitted two stages
  behind its compute (lag-2 wave), so it executes on DVE/ACT while the
  PE streams another strip's matmuls. PE should never wait on LN.
- LN stats: both s1 and s2 via fp8 DoubleRow matmuls (2 kt-chunks per MM,
  f32 PSUM accumulation). h and h^2 are produced on DVE during the j2
  mains (h^2 squared in fp16, rounded to fp8 only for the 1024-wide sum,
  so the rounding is averaged-out noise); stats matmuls never wait on the
  DVE. inv-std via DVE reciprocal + ACT Sqrt so every ACT func lives in
  one table (no 1.28us table swaps). Per-sample scale/shift vectors are
  partition-broadcast on the otherwise-idle GpSimd engine (no PE matmuls,
  no ACT copies, frees 3 PSUM banks for deeper main-evac buffering).
- gamma==1/beta==0 fast path (guaranteed by the reference's setup_inputs;
  build-time flag falls back to a full apply).
- Final layer evacuates straight to f32 and DMAs to the output.
"""

import sys

sys.path.insert(0, "/opt/trn_rl_repo")

import numpy as np

import ml_dtypes

import concourse.bass as bass
from concourse import bacc, mybir
import concourse.tile as tile
from concourse.bass_utils import run_bass_kernel_spmd

f32 = mybir.dt.float32
f16 = mybir.dt.float16
f8 = mybir.dt.float8e4
AF = mybir.ActivationFunctionType
Alu = mybir.AluOpType
DR = mybir.MatmulPerfMode.DoubleRow
F16 = np.float16
F8 = ml_dtypes.float8_e4m3

N_CORES = 8
DIM = 1024
KT = 8  # 1024 / 128 partition tiles
NL = 18
RANK = 32
GROUP = 16
BATCH = 32768
RPC = BATCH // N_CORES  # rows per core
NT = 512  # matmul moving free dim (one PSUM bank of fp32)
NSTRIP = RPC // NT
N_PASS = 3
LPP = NL // N_PASS  # layers resident per pass
SGRP = 4  # strips pipelined together (>=3 so the lag-2 LN wave works)
EPS = 1e-5
DEBUG_DR = False


def build_kernel(rows: int = RPC, apply_gb: bool = False):
    nc = bacc.Bacc()
    nstrip = rows // NT

    x_d = nc.declare_dram_parameter("x_t", [128, KT, rows], f16, False)
    w_d = nc.declare_dram_parameter("w_t", [NL, 128, KT, DIM], f16, False)
    bi_d = nc.declare_dram_parameter("bias_pp", [128, NL, KT], f32, False)
    ga_d = nc.declare_dram_parameter("gamma_pp", [128, 5, KT], f32, False)
    be_d = nc.declare_dram_parameter("beta_pp", [128, 5, KT], f32, False)
    onc_d = nc.declare_dram_parameter("ones_col", [128, 1], f16, False)
    onr_d = nc.declare_dram_parameter("ones_row", [1, 128], f16, False)
    on8_d = nc.declare_dram_parameter("ones_dr", [128, 2, 16], f8, False)
    y_d = nc.declare_dram_parameter("y_t", [128, KT, rows], f32, True)
    if DEBUG_DR:
        dbg_s2_d = nc.declare_dram_parameter("dbg_s2", [16, NT], f32, True)
        dbg_tin_d = nc.declare_dram_parameter("dbg_tin", [128, KT, NT], f16, True)

    with tile.TileContext(nc) as tc:
        with (
            tc.tile_pool(name="persist", bufs=1) as pp,
            tc.tile_pool(name="strips", bufs=1) as hp,
            tc.tile_pool(name="small", bufs=2) as sp,
            tc.tile_pool(name="ps_y", bufs=6, space="PSUM") as psy,
            tc.tile_pool(name="ps_st", bufs=2, space="PSUM") as pss,
            tc.tile_pool(name="rdram", bufs=1, space="DRAM") as dr,
        ):
            # persistent params: DMAs deferred until after the startup-
            # critical w0/tin transfers (each small DMA pays ~1us latency)
            bias_t = pp.tile([128, NL, KT], f32)
            gamma_t = pp.tile([128, 5, KT], f32)
            beta_t = pp.tile([128, 5, KT], f32)
            ones_c = pp.tile([128, 1], f16)
            ones_r = pp.tile([1, 128], f16)
            # DoubleRow stationary needs a 3D [K, 2, M] AP with middle
            # stride %16==0 -> M=16 columns of ones (all rows compute s2)
            ones_8 = pp.tile([128, 2, 16], f8)

            def load_params():
                nc.sync.dma_start(gamma_t[:, :, :], ga_d[:, :, :])
                nc.sync.dma_start(beta_t[:, :, :], be_d[:, :, :])
                nc.sync.dma_start(ones_c[:, :], onc_d[:, :])
                nc.sync.dma_start(ones_r[:, :], onr_d[:, :])
                nc.sync.dma_start(ones_8[:, :, :], on8_d[:, :, :])

            # 6 resident weight slots, reloaded once per pass
            w_sb = [
                pp.tile([128, KT, DIM], f16, name=f"w{i}") for i in range(LPP)
            ]
            # inter-pass hidden state (ping-pong)
            h_dram = [
                dr.tile([128, KT, rows], f16, tag=f"h{i}", name=f"hdram{i}")
                for i in range(2)
            ]

            for p in range(N_PASS):
                # w0 first so the first stage isn't stuck behind 12MB of
                # weight DMA; split per-kt so it spreads across DMA queues.
                # The rest queue after the first group's tins.
                for h in range(2):
                    hs = bass.ts(h, KT // 2)
                    nc.sync.dma_start(
                        w_sb[0][:, hs, :], w_d[p * LPP, :, hs, :]
                    )
                if p == 0:
                    nc.sync.dma_start(bias_t[:, :, :], bi_d[:, :, :])
                pending_w = list(range(1, LPP))
                src_d = x_d if p == 0 else h_dram[(p + 1) % 2]

                for g0 in range(0, nstrip, SGRP):
                    grp = list(range(g0, min(g0 + SGRP, nstrip)))
                    tins = {}
                    for s in grp:
                        t = hp.tile(
                            [128, KT, NT], f16, tag="tin", bufs=SGRP + 1
                        )
                        nc.sync.dma_start(t[:, :, :], src_d[:, :, bass.ts(s, NT)])
                        tins[s] = t
                    if p == 0 and g0 == 0:
                        load_params()
                    for i in pending_w:
                        nc.sync.dma_start(
                            w_sb[i][:, :, :], w_d[p * LPP + i, :, :, :]
                        )
                    pending_w = []
                    stats = {}

                    def do_stage(b2, s):
                        """Three matmul layers + (if LN) the stats matmuls."""
                        blk = 2 * p + b2
                        tin = tins[s]
                        tA = hp.tile([128, KT, NT], f16, tag="tA")
                        tB = hp.tile([128, KT, NT], f16, tag="tB")
                        hq8 = h8 = None
                        if blk < 5:
                            hq8 = sp.tile(
                                [128, KT, NT], f8, tag="hq8", bufs=2,
                                name=f"hq8_{p}_{s}_{b2}",
                            )
                            h8 = sp.tile(
                                [128, KT, NT], f8, tag="h8", bufs=1,
                                name=f"h8_{p}_{s}_{b2}",
                            )
                        for j in range(3):
                            li = 3 * b2 + j
                            l = p * LPP + li
                            src = tin if j == 0 else (tA if j == 1 else tB)
                            dst = tA if j == 0 else tB
                            for ot in range(KT):
                                ps = psy.tile([128, NT], f32, tag="y")
                                for kt in range(KT):
                                    nc.tensor.matmul(
                                        ps[:, :],
                                        lhsT=w_sb[li][:, kt, bass.ts(ot, 128)],
                                        rhs=src[:, kt, :],
                                        start=(kt == 0),
                                        stop=(kt == KT - 1),
                                    )
                                if j < 2:
                                    nc.scalar.activation(
                                        dst[:, ot, :],
                                        ps[:, :],
                                        AF.Relu,
                                        bias=bias_t[:, l, ot : ot + 1],
                                    )
                                elif blk == 5:
                                    # final layer: f32 out, straight to DRAM
                                    y32 = sp.tile([128, NT], f32, tag="y32")
                                    nc.vector.scalar_tensor_tensor(
                                        y32[:, :],
                                        ps[:, :],
                                        bias_t[:, l, ot : ot + 1],
                                        tin[:, ot, :],
                                        Alu.add,
                                        Alu.add,
                                    )
                                    nc.sync.dma_start(
                                        y_d[:, ot, bass.ts(s, NT)], y32[:, :]
                                    )
                                else:
                                    # h = (psum + bias) + r, in place on tin
                                    nc.vector.scalar_tensor_tensor(
                                        tin[:, ot, :],
                                        ps[:, :],
                                        bias_t[:, l, ot : ot + 1],
                                        tin[:, ot, :],
                                        Alu.add,
                                        Alu.add,
                                    )
                                    # square for LN stats, produced during
                                    # the j2 mains so stats MMs never wait
                                    if blk < 5:
                                        nc.vector.tensor_mul(
                                            hq8[:, ot, :],
                                            tin[:, ot, :],
                                            tin[:, ot, :],
                                        )
                                        nc.vector.tensor_copy(
                                            h8[:, ot, :], tin[:, ot, :]
                                        )
                        # LN stats: s1 = 1^T h (fp16, exact in f32 psum).
                        # s2 = 1^T h^2 with h^2 squared in fp16 precision but
                        # summed from an fp8 rounding via a DoubleRow matmul
                        # (2 kt-chunks per MM at 2x rate). The fp8 rounding of
                        # h^2 is unbiased noise, ~nil effect after the 1024-sum.
                        if blk < 5:
                            s1p = pss.tile([16, NT], f32, tag="st")
                            s2p = pss.tile([16, NT], f32, tag="st")
                            for k in range(KT // 2):
                                nc.tensor.matmul(
                                    s1p[:, :],
                                    lhsT=ones_8[:, :, :],
                                    rhs=h8[:, 2 * k : 2 * k + 2, :],
                                    start=(k == 0),
                                    stop=(k == KT // 2 - 1),
                                    perf_mode=DR,
                                )
                            for k in range(KT // 2):
                                nc.tensor.matmul(
                                    s2p[:, :],
                                    lhsT=ones_8[:, :, :],
                                    rhs=hq8[:, 2 * k : 2 * k + 2, :],
                                    start=(k == 0),
                                    stop=(k == KT // 2 - 1),
                                    perf_mode=DR,
                                )
                            if DEBUG_DR and p == 0 and b2 == 0 and s == 0:
                                s2d = psb.tile([16, NT], f32, tag="bc")
                                for k in range(KT // 2):
                                    nc.tensor.matmul(
                                        s2d[:, :],
                                        lhsT=ones_8[:, :, :],
                                        rhs=hq8[:, 2 * k : 2 * k + 2, :],
                                        start=(k == 0),
                                        stop=(k == KT // 2 - 1),
                                        perf_mode=DR,
                                    )
                                dbg = sp.tile([16, NT], f32, name="dbg_cp")
                                nc.vector.tensor_copy(dbg[:, :], s2d[:, :])
                                nc.sync.dma_start(dbg_s2_d[:, :], dbg[:, :])
                                nc.sync.dma_start(
                                    dbg_tin_d[:, :, :], tin[:, :, :]
                                )
                            # evacuate the tiny stats psums immediately so
                            # the banks recycle fast (frees budget for psy)
                            s1s = sp.tile([1, NT], f32, tag="s1s")
                            nc.vector.tensor_copy(s1s[:, :], s1p[0:1, :])
                            s2s = sp.tile([1, NT], f32, tag="s2s")
                            nc.vector.tensor_copy(s2s[:, :], s2p[0:1, :])
                            stats[(b2, s)] = (s1s, s2s)

                    bcast = {}

                    def fin_chain(b2, s):
                        """LN chain + partition-broadcast matmuls + evacs."""
                        blk = 2 * p + b2
                        if blk < 5:
                            s1p, s2p = stats.pop((b2, s))
                            m_sb = sp.tile([1, NT], f32, tag="m")
                            nc.vector.tensor_scalar(
                                m_sb[:, :], s1p[:, :], 1.0 / DIM, None, Alu.mult
                            )
                            msq = sp.tile([1, NT], f32, tag="msq")
                            nc.vector.tensor_mul(msq[:, :], m_sb[:, :], m_sb[:, :])
                            # var = s2/D - m^2
                            var_sb = sp.tile([1, NT], f32, tag="var")
                            nc.vector.scalar_tensor_tensor(
                                var_sb[:, :], s2p[:, :], 1.0 / DIM, msq[:, :],
                                Alu.mult, Alu.subtract,
                            )
                            # inv = sqrt(1/var); eps dropped (var >> 1e-5,
                            # relative effect < 1e-5). DVE reciprocal + ACT
                            # Sqrt keeps every ACT func in ONE table
                            # (sqrt_and_others) -> no 1.28us table swaps.
                            rvar = sp.tile([1, NT], f32, tag="rvar")
                            nc.vector.reciprocal(rvar[:, :], var_sb[:, :])
                            inv = sp.tile([1, NT], f16, tag="inv")
                            nc.scalar.activation(inv[:, :], rvar[:, :], AF.Sqrt)
                            mi = sp.tile([1, NT], f16, tag="mi")
                            nc.vector.tensor_mul(mi[:, :], m_sb[:, :], inv[:, :])
                            # partition-broadcast on the (idle) GpSimd
                            # engine: frees the PE bcast matmuls + ACT copies
                            A_sb = sp.tile([128, NT], f16, tag="A")
                            nc.gpsimd.partition_broadcast(A_sb[:, :], inv[:, :])
                            B_sb = sp.tile([128, NT], f16, tag="B")
                            nc.gpsimd.partition_broadcast(B_sb[:, :], mi[:, :])
                            bcast[(b2, s)] = (A_sb, B_sb)

                    def fin_apply(b2, s):
                        """LN apply in place on tin; inter-pass writeback."""
                        blk = 2 * p + b2
                        tin = tins[s]
                        if blk < 5:
                            A_sb, B_sb = bcast.pop((b2, s))
                            for kt in range(KT):
                                # h = h*inv - m*inv  (gamma=1, beta=0)
                                nc.vector.tensor_mul(
                                    tin[:, kt, :], tin[:, kt, :], A_sb[:, :]
                                )
                                nc.vector.tensor_sub(
                                    tin[:, kt, :], tin[:, kt, :], B_sb[:, :]
                                )
                                if apply_gb:
                                    nc.scalar.activation(
                                        tin[:, kt, :],
                                        tin[:, kt, :],
                                        AF.Identity,
                                        bias=beta_t[:, blk, kt : kt + 1],
                                        scale=gamma_t[:, blk, kt : kt + 1],
                                    )
                        # strip finished this pass: persist h for the next one
                        if b2 == 1 and p < N_PASS - 1:
                            nc.sync.dma_start(
                                h_dram[p % 2][:, :, bass.ts(s, NT)],
                                tin[:, :, :],
                            )

                    # lag-2 wave: stage i's LN finish is emitted after
                    # stage i+2's compute, so it overlaps other strips' mains.
                    # The flush pair is interleaved (chains before applies) so
                    # the second chain doesn't queue behind the first apply.
                    stages = [(b2, s) for b2 in range(2) for s in grp]
                    for i, (b2, s) in enumerate(stages):
                        do_stage(b2, s)
                        if i >= 2:
                            fin_chain(*stages[i - 2])
                            fin_apply(*stages[i - 2])
                    fin_chain(*stages[-2])
                    fin_chain(*stages[-1])
                    fin_apply(*stages[-2])
                    fin_apply(*stages[-1])

    nc.compile()
    return nc


def prep_inputs(x, wq, scales, bias, lora_a, lora_b, gamma, beta,
                rows_per_core=RPC):
    """Host-side prep: full dequant + LoRA fold in fp32, one fp16 rounding."""
    # W_eff[l] = (q - 8) * s + lb @ la   (layout [o, k])
    w_eff = (wq.astype(np.float32) - 8.0) * scales.reshape(
        NL, DIM, DIM // GROUP
    ).repeat(GROUP, axis=2)
    w_eff += np.einsum(
        "lor,lrk->lok", lora_b.astype(np.float32), lora_a.astype(np.float32)
    )
    # lhsT layout: w_t[l, p, kt, o] = W_eff[l, o, kt*128 + p]
    w_t = np.ascontiguousarray(
        w_eff.transpose(0, 2, 1).reshape(NL, KT, 128, DIM).transpose(0, 2, 1, 3)
    ).astype(F16)

    bias_pp = np.ascontiguousarray(
        bias.reshape(NL, KT, 128).transpose(2, 0, 1)
    ).astype(np.float32)
    gamma_pp = np.ascontiguousarray(
        gamma.reshape(5, KT, 128).transpose(2, 0, 1)
    ).astype(np.float32)
    beta_pp = np.ascontiguousarray(
        beta.reshape(5, KT, 128).transpose(2, 0, 1)
    ).astype(np.float32)

    shared = {
        "w_t": w_t, "bias_pp": bias_pp, "gamma_pp": gamma_pp,
        "beta_pp": beta_pp,
        "ones_col": np.ones((128, 1), F16),
        "ones_row": np.ones((1, 128), F16),
        "ones_dr": np.ones((128, 2, 16), F8),
    }
    in_maps = []
    for c in range(x.shape[0] // rows_per_core):
        xs = x[c * rows_per_core : (c + 1) * rows_per_core]  # [rows, 1024]
        x_t = np.ascontiguousarray(
            xs.T.reshape(KT, 128, rows_per_core).transpose(1, 0, 2)
        ).astype(F16)
        in_maps.append({"x_t": x_t, **shared})
    return in_maps


def unshard_output(results, rows_per_core=RPC):
    outs = []
    for r in results:
        y_t = np.asarray(r["y_t"]).reshape(128, KT, rows_per_core)
        outs.append(y_t.transpose(2, 1, 0).reshape(rows_per_core, DIM))
    return np.ascontiguousarray(np.concatenate(outs, axis=0), dtype=np.float32)


def kernel(x, wq, scales, bias, lora_a, lora_b, gamma, beta):
    x, wq, scales, bias, lora_a, lora_b, gamma, beta = (
        np.asarray(a) for a in (x, wq, scales, bias, lora_a, lora_b, gamma, beta)
    )
    apply_gb = not (np.all(gamma == 1.0) and np.all(beta == 0.0))
    nc = build_kernel(apply_gb=apply_gb)
    in_maps = prep_inputs(x, wq, scales, bias, lora_a, lora_b, gamma, beta)
    res = run_bass_kernel_spmd(nc, in_maps, list(range(N_CORES)))
    return unshard_output(res.results)


# revision 33
# speedup vs baseline: 1.0151x; 1.0151x over previous
"""TRN2 Bass kernel for nn_CustomQLoRABigNet: 6 blocks x (3 QLoRA linears),
ReLU, residual, LayerNorm. Data-parallel over 8 NeuronCores (4096 rows each).

v3 strategy (vs v2 baseline at 2.58ms):
- All weight prep happens on host: W_eff = (q-8)*s + lb@la computed in fp32
  and rounded ONCE to fp16. No dequant / LoRA-fold work on device at all
  (removes 288 fold matmuls + ~430 vector ops + 75MB scales DMA per core).
- fp16 activations/weights everywhere (same PE rate as bf16, 4x less
  rounding error -> large accuracy margin vs the 2e-2 gate).
- Pass/strip-major loop: 3 passes x 6 resident layers (96KB/partition).
  Within a pass each strip of 512 rows flows through all 6 layers using
  two scratch tiles (tA/tB) and an in-place carry tile (tIN) that holds
  the residual; no snapshot copies, no DRAM residual round-trips.
- Strips pipelined in groups of 4; the LayerNorm finish (stats chain,
  rank-1 broadcast matmuls, apply) for stage i is emitted two stages
  behind its compute (lag-2 wave), so it executes on DVE/ACT while the
  PE streams another strip's matmuls. PE should never wait on LN.
- LN stats: both s1 and s2 via fp8 DoubleRow matmuls (2 kt-chunks per MM,
  f32 PSUM accumulation). h and h^2 are produced on DVE during the j2
  mains (h^2 squared in fp16, rounded to fp8 only for the 1024-wide sum,
  so the rounding is averaged-out noise); stats matmuls never wait on the
  DVE. inv-std via DVE reciprocal + ACT Sqrt so every ACT func lives in
  one table (no 1.28us table swaps). Per-sample scale/shift vectors are
  partition-broadcast on the otherwise-idle GpSimd engine (no PE matmuls,
  no ACT copies, frees 3 PSUM banks for deeper main-evac buffering).
- gamma==1/beta==0 fast path (guaranteed by the reference's setup_inputs;
  build-time flag falls back to a full apply).
- Final layer evacuates straight to f32 and DMAs to the output.
"""

import sys

sys.path.insert(0, "/opt/trn_rl_repo")

import numpy as np

import ml_dtypes

import concourse.bass as bass
from concourse import bacc, mybir
import concourse.tile as tile
from concourse.bass_utils import run_bass_kernel_spmd

f32 = mybir.dt.float32
f16 = mybir.dt.float16
f8 = mybir.dt.float8e4
AF = mybir.ActivationFunctionType
Alu = mybir.AluOpType
DR = mybir.MatmulPerfMode.DoubleRow
F16 = np.float16
F8 = ml_dtypes.float8_e4m3

N_CORES = 8
DIM = 1024
KT = 8  # 1024 / 128 partition tiles
NL = 18
RANK = 32
GROUP = 16
BATCH = 32768
RPC = BATCH // N_CORES  # rows per core
NT = 512  # matmul moving free dim (one PSUM bank of fp32)
NSTRIP = RPC // NT
N_PASS = 3
LPP = NL // N_PASS  # layers resident per pass
SGRP = 4  # strips pipelined together (>=3 so the lag-2 LN wave works)
EPS = 1e-5
DEBUG_DR = False


def build_kernel(rows: int = RPC, apply_gb: bool = False):
    nc = bacc.Bacc()
    nstrip = rows // NT

    x_d = nc.declare_dram_parameter("x_t", [128, KT, rows], f16, False)
    w_d = nc.declare_dram_parameter("w_t", [NL, 128, KT, DIM], f16, False)
    bi_d = nc.declare_dram_parameter("bias_pp", [128, NL, KT], f32, False)
    ga_d = nc.declare_dram_parameter("gamma_pp", [128, 5, KT], f32, False)
    be_d = nc.declare_dram_parameter("beta_pp", [128, 5, KT], f32, False)
    on8_d = nc.declare_dram_parameter("ones_dr", [128, 2, 16], f8, False)
    y_d = nc.declare_dram_parameter("y_t", [128, KT, rows], f32, True)
    if DEBUG_DR:
        dbg_s2_d = nc.declare_dram_parameter("dbg_s2", [16, NT], f32, True)
        dbg_tin_d = nc.declare_dram_parameter("dbg_tin", [128, KT, NT], f16, True)

    with tile.TileContext(nc) as tc:
        with (
            tc.tile_pool(name="persist", bufs=1) as pp,
            tc.tile_pool(name="strips", bufs=1) as hp,
            tc.tile_pool(name="small", bufs=2) as sp,
            tc.tile_pool(name="ps_y", bufs=6, space="PSUM") as psy,
            tc.tile_pool(name="ps_st", bufs=2, space="PSUM") as pss,
            tc.tile_pool(name="rdram", bufs=1, space="DRAM") as dr,
        ):
            # persistent params: DMAs deferred until after the startup-
            # critical w0/tin transfers (each small DMA pays ~1us latency)
            bias_t = pp.tile([128, NL, KT], f32)
            gamma_t = pp.tile([128, 5, KT], f32)
            beta_t = pp.tile([128, 5, KT], f32)
            # DoubleRow stationary needs a 3D [K, 2, M] AP with middle
            # stride %16==0 -> M=16 columns of ones (all rows compute s2)
            ones_8 = pp.tile([128, 2, 16], f8)

            def load_params():
                nc.sync.dma_start(ones_8[:, :, :], on8_d[:, :, :])
                if apply_gb:
                    nc.sync.dma_start(gamma_t[:, :, :], ga_d[:, :, :])
                    nc.sync.dma_start(beta_t[:, :, :], be_d[:, :, :])

            # 6 resident weight slots, reloaded once per pass
            w_sb = [
                pp.tile([128, KT, DIM], f16, name=f"w{i}") for i in range(LPP)
            ]
            # inter-pass hidden state (ping-pong)
            h_dram = [
                dr.tile([128, KT, rows], f16, tag=f"h{i}", name=f"hdram{i}")
                for i in range(2)
            ]

            for p in range(N_PASS):
                # w0 first so the first stage isn't stuck behind 12MB of
                # weight DMA; split per-kt so it spreads across DMA queues.
                # The rest queue after the first group's tins.
                if p == 0:
                    tin0 = hp.tile(
                        [128, KT, NT], f16, tag="tin", bufs=SGRP + 1
                    )
                    nc.sync.dma_start(tin0[:, :, :], x_d[:, :, 0:NT])
                for h in range(2):
                    hs = bass.ts(h, KT // 2)
                    nc.sync.dma_start(
                        w_sb[0][:, hs, :], w_d[p * LPP, :, hs, :]
                    )
                if p == 0:
                    nc.sync.dma_start(bias_t[:, :, :], bi_d[:, :, :])
                pending_w = list(range(1, LPP))
                src_d = x_d if p == 0 else h_dram[(p + 1) % 2]

                for g0 in range(0, nstrip, SGRP):
                    grp = list(range(g0, min(g0 + SGRP, nstrip)))
                    tins = {}
                    for s in grp:
                        if p == 0 and s == 0:
                            tins[s] = tin0
                            continue
                        t = hp.tile(
                            [128, KT, NT], f16, tag="tin", bufs=SGRP + 1
                        )
                        nc.sync.dma_start(t[:, :, :], src_d[:, :, bass.ts(s, NT)])
                        tins[s] = t
                    if p == 0 and g0 == 0:
                        load_params()
                    for i in pending_w:
                        nc.sync.dma_start(
                            w_sb[i][:, :, :], w_d[p * LPP + i, :, :, :]
                        )
                    pending_w = []
                    stats = {}

                    def do_stage(b2, s):
                        """Three matmul layers + (if LN) the stats matmuls."""
                        blk = 2 * p + b2
                        tin = tins[s]
                        tA = hp.tile([128, KT, NT], f16, tag="tA")
                        tB = hp.tile([128, KT, NT], f16, tag="tB")
                        hq8 = h8 = None
                        if blk < 5:
                            hq8 = sp.tile(
                                [128, KT, NT], f8, tag="hq8", bufs=2,
                                name=f"hq8_{p}_{s}_{b2}",
                            )
                            h8 = sp.tile(
                                [128, KT, NT], f8, tag="h8", bufs=1,
                                name=f"h8_{p}_{s}_{b2}",
                            )
                        for j in range(3):
                            li = 3 * b2 + j
                            l = p * LPP + li
                            src = tin if j == 0 else (tA if j == 1 else tB)
                            dst = tA if j == 0 else tB
                            for ot in range(KT):
                                ps = psy.tile([128, NT], f32, tag="y")
                                for kt in range(KT):
                                    nc.tensor.matmul(
                                        ps[:, :],
                                        lhsT=w_sb[li][:, kt, bass.ts(ot, 128)],
                                        rhs=src[:, kt, :],
                                        start=(kt == 0),
                                        stop=(kt == KT - 1),
                                    )
                                if j < 2:
                                    nc.scalar.activation(
                                        dst[:, ot, :],
                                        ps[:, :],
                                        AF.Relu,
                                        bias=bias_t[:, l, ot : ot + 1],
                                    )
                                elif blk == 5:
                                    # final layer: f32 out, straight to DRAM
                                    y32 = sp.tile([128, NT], f32, tag="y32")
                                    nc.vector.scalar_tensor_tensor(
                                        y32[:, :],
                                        ps[:, :],
                                        bias_t[:, l, ot : ot + 1],
                                        tin[:, ot, :],
                                        Alu.add,
                                        Alu.add,
                                    )
                                    nc.sync.dma_start(
                                        y_d[:, ot, bass.ts(s, NT)], y32[:, :]
                                    )
                                else:
                                    # h = (psum + bias) + r, in place on tin
                                    nc.vector.scalar_tensor_tensor(
                                        tin[:, ot, :],
                                        ps[:, :],
                                        bias_t[:, l, ot : ot + 1],
                                        tin[:, ot, :],
                                        Alu.add,
                                        Alu.add,
                                    )
                                    # square for LN stats, produced during
                                    # the j2 mains so stats MMs never wait
                                    if blk < 5:
                                        nc.vector.tensor_mul(
                                            hq8[:, ot, :],
                                            tin[:, ot, :],
                                            tin[:, ot, :],
                                        )
                                        nc.vector.tensor_copy(
                                            h8[:, ot, :], tin[:, ot, :]
                                        )
                        # LN stats: s1 = 1^T h (fp16, exact in f32 psum).
                        # s2 = 1^T h^2 with h^2 squared in fp16 precision but
                        # summed from an fp8 rounding via a DoubleRow matmul
                        # (2 kt-chunks per MM at 2x rate). The fp8 rounding of
                        # h^2 is unbiased noise, ~nil effect after the 1024-sum.
                        if blk < 5:
                            s1p = pss.tile([16, NT], f32, tag="st")
                            s2p = pss.tile([16, NT], f32, tag="st")
                            for k in range(KT // 2):
                                nc.tensor.matmul(
                                    s1p[:, :],
                                    lhsT=ones_8[:, :, :],
                                    rhs=h8[:, 2 * k : 2 * k + 2, :],
                                    start=(k == 0),
                                    stop=(k == KT // 2 - 1),
                                    perf_mode=DR,
                                )
                            for k in range(KT // 2):
                                nc.tensor.matmul(
                                    s2p[:, :],
                                    lhsT=ones_8[:, :, :],
                                    rhs=hq8[:, 2 * k : 2 * k + 2, :],
                                    start=(k == 0),
                                    stop=(k == KT // 2 - 1),
                                    perf_mode=DR,
                                )
                            if DEBUG_DR and p == 0 and b2 == 0 and s == 0:
                                s2d = psb.tile([16, NT], f32, tag="bc")
                                for k in range(KT // 2):
                                    nc.tensor.matmul(
                                        s2d[:, :],
                                        lhsT=ones_8[:, :, :],
                                        rhs=hq8[:, 2 * k : 2 * k + 2, :],
                                        start=(k == 0),
                                        stop=(k == KT // 2 - 1),
                                        perf_mode=DR,
                                    )
                                dbg = sp.tile([16, NT], f32, name="dbg_cp")
                                nc.vector.tensor_copy(dbg[:, :], s2d[:, :])
                                nc.sync.dma_start(dbg_s2_d[:, :], dbg[:, :])
                                nc.sync.dma_start(
                                    dbg_tin_d[:, :, :], tin[:, :, :]
                                )
                            # evacuate the tiny stats psums immediately so
                            # the banks recycle fast (frees budget for psy)
                            s1s = sp.tile([1, NT], f32, tag="s1s")
                            nc.vector.tensor_copy(s1s[:, :], s1p[0:1, :])
                            s2s = sp.tile([1, NT], f32, tag="s2s")
                            nc.vector.tensor_copy(s2s[:, :], s2p[0:1, :])
                            stats[(b2, s)] = (s1s, s2s)

                    bcast = {}

                    def fin_chain(b2, s):
                        """LN chain + partition-broadcast matmuls + evacs."""
                        blk = 2 * p + b2
                        if blk < 5:
                            s1p, s2p = stats.pop((b2, s))
                            m_sb = sp.tile([1, NT], f32, tag="m")
                            nc.vector.tensor_scalar(
                                m_sb[:, :], s1p[:, :], 1.0 / DIM, None, Alu.mult
                            )
                            msq = sp.tile([1, NT], f32, tag="msq")
                            nc.vector.tensor_mul(msq[:, :], m_sb[:, :], m_sb[:, :])
                            # var = s2/D - m^2
                            var_sb = sp.tile([1, NT], f32, tag="var")
                            nc.vector.scalar_tensor_tensor(
                                var_sb[:, :], s2p[:, :], 1.0 / DIM, msq[:, :],
                                Alu.mult, Alu.subtract,
                            )
                            # inv = sqrt(1/var); eps dropped (var >> 1e-5,
                            # relative effect < 1e-5). DVE reciprocal + ACT
                            # Sqrt keeps every ACT func in ONE table
                            # (sqrt_and_others) -> no 1.28us table swaps.
                            rvar = sp.tile([1, NT], f32, tag="rvar")
                            nc.vector.reciprocal(rvar[:, :], var_sb[:, :])
                            inv = sp.tile([1, NT], f16, tag="inv")
                            nc.scalar.activation(inv[:, :], rvar[:, :], AF.Sqrt)
                            mi = sp.tile([1, NT], f16, tag="mi")
                            nc.vector.tensor_mul(mi[:, :], m_sb[:, :], inv[:, :])
                            # partition-broadcast on the (idle) GpSimd
                            # engine: frees the PE bcast matmuls + ACT copies
                            A_sb = sp.tile([128, NT], f16, tag="A")
                            nc.gpsimd.partition_broadcast(A_sb[:, :], inv[:, :])
                            B_sb = sp.tile([128, NT], f16, tag="B")
                            nc.gpsimd.partition_broadcast(B_sb[:, :], mi[:, :])
                            bcast[(b2, s)] = (A_sb, B_sb)

                    def fin_apply(b2, s):
                        """LN apply in place on tin; inter-pass writeback."""
                        blk = 2 * p + b2
                        tin = tins[s]
                        if blk < 5:
                            A_sb, B_sb = bcast.pop((b2, s))
                            for kt in range(KT):
                                # h = h*inv - m*inv  (gamma=1, beta=0)
                                nc.vector.tensor_mul(
                                    tin[:, kt, :], tin[:, kt, :], A_sb[:, :]
                                )
                                nc.vector.tensor_sub(
                                    tin[:, kt, :], tin[:, kt, :], B_sb[:, :]
                                )
                                if apply_gb:
                                    nc.scalar.activation(
                                        tin[:, kt, :],
                                        tin[:, kt, :],
                                        AF.Identity,
                                        bias=beta_t[:, blk, kt : kt + 1],
                                        scale=gamma_t[:, blk, kt : kt + 1],
                                    )
                        # strip finished this pass: persist h for the next one
                        if b2 == 1 and p < N_PASS - 1:
                            nc.sync.dma_start(
                                h_dram[p % 2][:, :, bass.ts(s, NT)],
                                tin[:, :, :],
                            )

                    # lag-2 wave: stage i's LN finish is emitted after
                    # stage i+2's compute, so it overlaps other strips' mains.
                    # The flush pair is interleaved (chains before applies) so
                    # the second chain doesn't queue behind the first apply.
                    stages = [(b2, s) for b2 in range(2) for s in grp]
                    for i, (b2, s) in enumerate(stages):
                        do_stage(b2, s)
                        if i >= 2:
                            fin_chain(*stages[i - 2])
                            fin_apply(*stages[i - 2])
                    fin_chain(*stages[-2])
                    fin_chain(*stages[-1])
                    fin_apply(*stages[-2])
                    fin_apply(*stages[-1])

    nc.compile()
    return nc


def prep_inputs(x, wq, scales, bias, lora_a, lora_b, gamma, beta,
                rows_per_core=RPC):
    """Host-side prep: full dequant + LoRA fold in fp32, one fp16 rounding."""
    # W_eff[l] = (q - 8) * s + lb @ la   (layout [o, k])
    w_eff = (wq.astype(np.float32) - 8.0) * scales.reshape(
        NL, DIM, DIM // GROUP
    ).repeat(GROUP, axis=2)
    w_eff += np.einsum(
        "lor,lrk->lok", lora_b.astype(np.float32), lora_a.astype(np.float32)
    )
    # lhsT layout: w_t[l, p, kt, o] = W_eff[l, o, kt*128 + p]
    w_t = np.ascontiguousarray(
        w_eff.transpose(0, 2, 1).reshape(NL, KT, 128, DIM).transpose(0, 2, 1, 3)
    ).astype(F16)

    bias_pp = np.ascontiguousarray(
        bias.reshape(NL, KT, 128).transpose(2, 0, 1)
    ).astype(np.float32)
    gamma_pp = np.ascontiguousarray(
        gamma.reshape(5, KT, 128).transpose(2, 0, 1)
    ).astype(np.float32)
    beta_pp = np.ascontiguousarray(
        beta.reshape(5, KT, 128).transpose(2, 0, 1)
    ).astype(np.float32)

    shared = {
        "w_t": w_t, "bias_pp": bias_pp, "gamma_pp": gamma_pp,
        "beta_pp": beta_pp,
        "ones_dr": np.ones((128, 2, 16), F8),
    }
    in_maps = []
    for c in range(x.shape[0] // rows_per_core):
        xs = x[c * rows_per_core : (c + 1) * rows_per_core]  # [rows, 1024]
        x_t = np.ascontiguousarray(
            xs.T.reshape(KT, 128, rows_per_core).transpose(1, 0, 2)
        ).astype(F16)
        in_maps.append({"x_t": x_t, **shared})
    return in_maps


def unshard_output(results, rows_per_core=RPC):
    outs = []
    for r in results:
        y_t = np.asarray(r["y_t"]).reshape(128, KT, rows_per_core)
        outs.append(y_t.transpose(2, 1, 0).reshape(rows_per_core, DIM))
    return np.ascontiguousarray(np.concatenate(outs, axis=0), dtype=np.float32)


def kernel(x, wq, scales, bias, lora_a, lora_b, gamma, beta):
    x, wq, scales, bias, lora_a, lora_b, gamma, beta = (
        np.asarray(a) for a in (x, wq, scales, bias, lora_a, lora_b, gamma, beta)
    )
    apply_gb = not (np.all(gamma == 1.0) and np.all(beta == 0.0))
    nc = build_kernel(apply_gb=apply_gb)
    in_maps = prep_inputs(x, wq, scales, bias, lora_a, lora_b, gamma, beta)
    res = run_bass_kernel_spmd(nc, in_maps, list(range(N_CORES)))
    return unshard_output(res.results)
